# revision 30
# baseline (speedup 1.0000x reference)
"""CondConv2d Trainium2 kernel.

Math: per-sample conv kernel = routing-weighted sum of 8 expert kernels,
then a 3x3 (pad 1, stride 1) conv per sample, plus a routed bias.

MODE="wino": 1D Winograd F(2,3) along the height axis. Per sample the
conv is computed over 28 row-pair tiles t (output rows 2t, 2t+1):
    V0 = r[2t]-r[2t+2], V1 = r[2t+1]+r[2t+2],
    V2 = r[2t+2]-r[2t+1], V3 = r[2t+1]-r[2t+3]      (rows of padded x)
    m_i = sum_dx U[i,dx]^T V_i[.., dx:dx+56]         (PE, 12 matmuls/blk)
    y[2t]   = m0+m1+m2 + bias
    y[2t+1] = m1-m2-m3 + bias
with U0=w0, U1=(w0+w1+w2)/2, U2=(w0-w1+w2)/2, U3=w2 (host, per dx slab).
This does the 9-tap conv in 12 matmul-streams per 16 output rows instead
of 18 -> PE cycles drop 9*3136 -> 6*3136 per sample (94us -> 63us/core).
Forward transform runs on gpsimd (bf16 adds), the inverse runs on
vector (scalar_tensor_tensor with fused per-partition bias) plus one
scalar-engine PSUM->SBUF copy of m1 per block.

MODE="direct": the original 9-accumulating-matmul direct conv.

Shared structure:
  - Host computes per-sample combined (transformed) kernels, packs them
    with zero-padded x into one bf16 tensor per sample; b_mix as [oc,bs].
  - Data-parallel over batch: 8 samples per core x 8 cores.
  - PE warm-up matmuls un-throttle the HAM clock gate during head DMA.

Hardcoded shapes: x[64,128,56,56] f32, routing_weight[64,8] f32,
weight[8,128,128,3,3] f32, bias[8,128] f32 -> out[64,128,56,56] f32.
"""

import os

import numpy as np

N_CORES = 8
BS, CIN, H, W = 64, 128, 56, 56
KEXP, OC = 8, 128
P = BS // N_CORES  # samples per core
HP, WP = H + 2, W + 2
XSZ = HP * WP      # padded x free size per partition
NT = H // 2        # 28 row-pair tiles per sample
USZ = 4 * 3 * OC   # wino weight slabs (i, dx, oc)
# packed free dim: [U | V1[0:16] | ev (29 rows) | V3[0:16] | V2[16:28] | V1[16:28] | V3[16:28]]
PKSZ_W = USZ + 29 * WP + 2 * 16 * WP + 3 * 12 * WP

# direct mode sizes
WSZ = 3 * 3 * OC
PKSZ_D = WSZ + XSZ
RB = 8
NBLK = H // RB

MODE = "wino"      # "wino" or "direct"
N_WARM = 42        # warm-ups bridge until the first real matmul (~9.8us)
                   # so the HAM ramp is not reset by a PE idle gap

_CACHE = {}


def _build_nc(mode):
    if mode in ("bf16", "f32r", "f32"):  # legacy arg from test.py SIM path
        mode = MODE
    import concourse.bacc as bacc
    import concourse.mybir as mybir
    import concourse.tile as tile

    if mode == "wino":
        return _build_wino(bacc, mybir, tile)
    return _build_direct(bacc, mybir, tile)


def _build_wino(bacc, mybir, tile):
    f32 = mybir.dt.float32
    bf16 = mybir.dt.bfloat16
    Alu = mybir.AluOpType
    Act = mybir.ActivationFunctionType

    nc = bacc.Bacc()
    xw = nc.dram_tensor("xw", [P, CIN, PKSZ_W], bf16, kind="ExternalInput")
    bm = nc.dram_tensor("bm", [OC, P], f32, kind="ExternalInput")
    # even rows (y0) and odd rows (y1) in separate planes; host interleaves
    out = nc.dram_tensor("out", [P, OC, 2, NT, W], bf16, kind="ExternalOutput")

    # free-dim layout offsets (see _host_prep)
    O_V1A = USZ
    O_EV = O_V1A + 16 * WP
    O_V3A = O_EV + 29 * WP
    O_V2B = O_V3A + 16 * WP
    O_V1B = O_V2B + 12 * WP
    O_V3B = O_V1B + 12 * WP

    BLOCKS = [(0, 8), (8, 8), (16, 8), (24, 4)]

    with tile.TileContext(nc) as tc:
        with (
            tc.tile_pool(name="xp", bufs=3) as xp,
            tc.tile_pool(name="vp", bufs=3) as vp,
            tc.tile_pool(name="op", bufs=8) as op,
            tc.tile_pool(name="sp", bufs=4) as sp,
            tc.tile_pool(name="bp", bufs=1) as bp,
            tc.tile_pool(name="ps", bufs=8, space="PSUM") as psp,
        ):
            bmt = bp.tile([OC, P], f32)

            # PE warm-up (HAM un-throttle) on gpsimd-memset scratch
            scr = bp.tile([OC, 192], bf16)
            nc.gpsimd.memset(scr, 0.0)
            scrp = psp.tile([OC, 64], f32, tag="pst", name="scrp")
            for i in range(N_WARM):
                nc.tensor.matmul(
                    out=scrp[:, :],
                    lhsT=scr[:, 0:128],
                    rhs=scr[:, 128:192],
                    start=True,
                    stop=True,
                    skip_group_check=True,
                )

            xwt = [None] * P
            view = [None] * P
            vtile = [None] * P
            v2tile = [None] * P

            def dma_in(b):
                t = xp.tile([CIN, PKSZ_W], bf16, tag="xwt", name=f"xwt{b}")
                xwt[b] = t
                if b == 0:
                    # head: one sync chunk covers all of block 0's m1
                    # needs (U lhsT + V1a rhs + ev rows 0..8); V3a on the
                    # scalar ring, tail slabs on the gpsimd ring
                    c1 = O_EV + 9 * WP
                    nc.sync.dma_start(out=t[:, :c1], in_=xw[b][:, :c1])
                    nc.sync.dma_start(out=bmt, in_=bm[:, :])
                    nc.sync.dma_start(out=t[:, c1:O_V3A],
                                      in_=xw[b][:, c1:O_V3A])
                    nc.scalar.dma_start(out=t[:, O_V3A:O_V2B],
                                        in_=xw[b][:, O_V3A:O_V2B])
                    nc.gpsimd.dma_start(out=t[:, O_V2B:],
                                        in_=xw[b][:, O_V2B:])
                else:
                    nc.sync.dma_start(out=t[:, :O_V2B],
                                      in_=xw[b][:, :O_V2B])
                    nc.scalar.dma_start(out=t[:, O_V2B:],
                                        in_=xw[b][:, O_V2B:])
                view[b] = dict(
                    ut=t[:, :USZ].rearrange(
                        "p (i dx oc) -> p i dx oc", i=4, dx=3),
                    v1a=t[:, O_V1A:O_EV].rearrange("p (r w) -> p r w", w=WP),
                    ev=t[:, O_EV:O_V3A].rearrange("p (r w) -> p r w", w=WP),
                    v3a=t[:, O_V3A:O_V2B].rearrange("p (r w) -> p r w", w=WP),
                    v2b=t[:, O_V2B:O_V1B].rearrange("p (r w) -> p r w", w=WP),
                    v1b=t[:, O_V1B:O_V3B].rearrange("p (r w) -> p r w", w=WP),
                    v3b=t[:, O_V3B:].rearrange("p (r w) -> p r w", w=WP),
                )
                vtile[b] = vp.tile([CIN, NT, WP], bf16, tag="vt",
                                   name=f"vt{b}")
                v2tile[b] = vp.tile([CIN, 16, WP], bf16, tag="v2t",
                                    name=f"v2t{b}")

            def fwd_v0(b, ts, te):
                # V0 = ev[t] - ev[t+1]  (gpsimd)
                ev = view[b]["ev"]
                nc.gpsimd.tensor_sub(
                    vtile[b][:, ts:te, :], ev[:, ts:te, :],
                    ev[:, ts + 1 : te + 1, :])

            def fwd_v2(b, ts, te):
                # V2 = 2*ev[t+1] - V1[t]  (vector; tiles 16..28 shipped)
                ev = view[b]["ev"]
                v1 = view[b]["v1a"]
                nc.vector.scalar_tensor_tensor(
                    out=v2tile[b][:, ts:te, :],
                    in0=ev[:, ts + 1 : te + 1, :], scalar=2.0,
                    in1=v1[:, ts:te, :],
                    op0=Alu.mult, op1=Alu.subtract)

            def rhs(b, i, t0, tn, dx):
                if i == 0:
                    return vtile[b][:, t0 : t0 + tn, dx : dx + W]
                if i == 2:
                    if t0 < 16:
                        return v2tile[b][:, t0 : t0 + tn, dx : dx + W]
                    return view[b]["v2b"][:, t0 - 16 : t0 - 16 + tn,
                                          dx : dx + W]
                key = ("v1a" if t0 < 16 else "v1b") if i == 1 else (
                    "v3a" if t0 < 16 else "v3b")
                o = 0 if t0 < 16 else 16
                return view[b][key][:, t0 - o : t0 - o + tn, dx : dx + W]

            def mms(b, blk, t0, tn, iorder):
                ut = view[b]["ut"]
                pst = {}
                for i in iorder:
                    ps = psp.tile([OC, tn, W], f32, tag="pst",
                                  name=f"pst{b}_{blk}_{i}")
                    pst[i] = ps
                    for dx in range(3):
                        nc.tensor.matmul(
                            out=ps[:, :, :],
                            lhsT=ut[:, i, dx, :],
                            rhs=rhs(b, i, t0, tn, dx),
                            start=(dx == 0),
                            stop=(dx == 2),
                            skip_group_check=True,
                        )
                return pst

            pending_out = [None]

            def flush_out():
                if pending_out[0] is not None:
                    d0, s0, d1, s1 = pending_out[0]
                    nc.scalar.dma_start(out=d0, in_=s0)
                    nc.sync.dma_start(out=d1, in_=s1)
                    pending_out[0] = None

            def scalar_ops(b, blk, tn, pst, tiles):
                # t1b = m1 + bias; m2f = copy(m2)   (scalar, PSUM reads)
                t1b = sp.tile([OC, tn, W], bf16, tag="t1b",
                              name=f"t1b_{b}_{blk}")
                m2f = sp.tile([OC, tn, W], bf16, tag="m2f",
                              name=f"m2f_{b}_{blk}")
                nc.scalar.activation(out=t1b, in_=pst[1][:, :, :],
                                     func=Act.Identity,
                                     bias=bmt[:, b : b + 1], scale=1.0)
                nc.scalar.activation(out=m2f, in_=pst[2][:, :, :],
                                     func=Act.Copy)
                tiles["t1b"], tiles["m2f"] = t1b, m2f

            def gpsimd_u(b, blk, tn, tiles):
                uf = sp.tile([OC, tn, W], bf16, tag="uf", name=f"u_{b}_{blk}")
                nc.gpsimd.tensor_sub(uf, tiles["t1b"], tiles["m2f"])
                tiles["uf"] = uf

            def vector_evict(b, blk, t0, tn, pst, tiles):
                s1f = sp.tile([OC, tn, W], bf16, tag="s1f",
                              name=f"s1_{b}_{blk}")
                y0t = op.tile([OC, tn, W], bf16, tag="y0t",
                              name=f"y0_{b}_{blk}")
                y1t = op.tile([OC, tn, W], bf16, tag="y1t",
                              name=f"y1_{b}_{blk}")
                nc.vector.tensor_add(s1f, tiles["t1b"], tiles["m2f"])
                nc.vector.tensor_add(y0t, pst[0][:, :, :], s1f)
                nc.vector.tensor_sub(y1t, tiles["uf"], pst[3][:, :, :])
                pending_out[0] = (out[b][:, 0, t0 : t0 + tn, :], y0t,
                                  out[b][:, 1, t0 : t0 + tn, :], y1t)

            dma_in(0)
            for b in range(P):
                if b + 1 < P:
                    dma_in(b + 1)
                blocks = BLOCKS
                nblk = len(blocks)
                if b == 0:
                    # head: the PE first streams the host-shipped V1/V3
                    # slabs of blocks 0+1 while the device computes V0/V2
                    fwd_v0(0, 0, 8)
                    fwd_v2(0, 0, 8)
                    fwd_v0(0, 8, 16)
                    fwd_v2(0, 8, 16)
                    fwd_v0(0, 16, 28)
                    psts = [dict() for _ in blocks]
                    for iorder in ((1, 3), (0, 2)):
                        for blk in (0, 1):
                            t0, tn = blocks[blk]
                            psts[blk].update(mms(0, blk, t0, tn, iorder))
                    for blk in (2, 3):
                        t0, tn = blocks[blk]
                        psts[blk] = mms(0, blk, t0, tn, (1, 2, 0, 3))
                else:
                    psts = [mms(b, blk, t0, tn, (1, 2, 0, 3))
                            for blk, (t0, tn) in enumerate(blocks)]
                tiles = [dict() for _ in range(nblk)]
                last = b == P - 1
                for blk, (t0, tn) in enumerate(blocks):
                    scalar_ops(b, blk, tn, psts[blk], tiles[blk])
                    flush_out()  # previous block's output on scalar ring
                    gpsimd_u(b, blk, tn, tiles[blk])
                    # interleave next-sample fwd into engine queues mid-way
                    if b + 1 < P:
                        if blk == 1:
                            fwd_v0(b + 1, 0, 16)
                        elif blk == 2:
                            fwd_v0(b + 1, 16, 28)
                            fwd_v2(b + 1, 0, 8)
                        elif blk == nblk - 1:
                            fwd_v2(b + 1, 8, 16)
                    vector_evict(b, blk, t0, tn, psts[blk], tiles[blk])
                    if last and blk == nblk - 1:
                        flush_out()
    _thin_engine_sem(nc, "PE_")
    nc.finalize()
    return nc


def _build_direct(bacc, mybir, tile):
    f32 = mybir.dt.float32
    bf16 = mybir.dt.bfloat16

    nc = bacc.Bacc()
    xw = nc.dram_tensor("xw", [P, CIN, PKSZ_D], bf16, kind="ExternalInput")
    bm = nc.dram_tensor("bm", [OC, P], f32, kind="ExternalInput")
    out = nc.dram_tensor("out", [P, OC, H, W], bf16, kind="ExternalOutput")

    taps = [(dy, dx) for dy in range(3) for dx in range(3)]

    with tile.TileContext(nc) as tc:
        with (
            tc.tile_pool(name="xp", bufs=4) as xp,
            tc.tile_pool(name="op", bufs=6) as op,
            tc.tile_pool(name="bp", bufs=1) as bp,
            tc.tile_pool(name="ps", bufs=8, space="PSUM") as psp,
        ):
            bmt = bp.tile([OC, P], f32)

            def evict(pst, obt_name, b, r0, rows):
                obt = op.tile([OC, rows, W], bf16, tag="obt", name=obt_name)
                nc.scalar.activation(
                    out=obt,
                    in_=pst[:, :, :],
                    func=mybir.ActivationFunctionType.Identity,
                    bias=bmt[:, b : b + 1],
                    scale=1.0,
                )
                nc.sync.dma_start(out=out[b][:, r0 : r0 + rows, :], in_=obt)

            scr = bp.tile([OC, 192], bf16)
            nc.gpsimd.memset(scr, 0.0)
            scrp = psp.tile([OC, 64], f32, tag="pst", name="scrp")
            for i in range(62):
                nc.tensor.matmul(
                    out=scrp[:, :],
                    lhsT=scr[:, 0:128],
                    rhs=scr[:, 128:192],
                    start=True,
                    stop=True,
                    skip_group_check=True,
                )

            for b in range(P):
                xwt = xp.tile([CIN, PKSZ_D], bf16)
                if b == 0:
                    c0 = WSZ + 8 * WP
                    c1 = WSZ + 10 * WP
                    c2 = WSZ + 26 * WP
                    nc.sync.dma_start(out=xwt[:, :c0], in_=xw[b][:, :c0])
                    nc.sync.dma_start(out=xwt[:, c0:c1], in_=xw[b][:, c0:c1])
                    nc.sync.dma_start(out=xwt[:, c1:c2], in_=xw[b][:, c1:c2])
                    nc.sync.dma_start(out=xwt[:, c2:], in_=xw[b][:, c2:])
                    nc.sync.dma_start(out=bmt, in_=bm[:, :])
                else:
                    nc.sync.dma_start(out=xwt, in_=xw[b])
                wt = xwt[:, :WSZ].rearrange(
                    "p (kh kw oc) -> p kh kw oc", kh=3, kw=3
                )
                xt = xwt[:, WSZ:].rearrange("p (h w) -> p h w", h=HP)
                if b == P - 1:
                    blocks = [(i * RB, RB) for i in range(NBLK - 1)]
                    blocks += [(48, 4), (52, 4)]
                else:
                    blocks = [(i * RB, RB) for i in range(NBLK)]
                pst = [
                    psp.tile([OC, rows, W], f32, tag="pst", name=f"pst{b}_{i}")
                    for i, (r0, rows) in enumerate(blocks)
                ]
                for blk, (r0, rows) in enumerate(blocks):
                    for it, (dy, dx) in enumerate(taps):
                        nc.tensor.matmul(
                            out=pst[blk][:, :, :],
                            lhsT=wt[:, dy, dx, :],
                            rhs=xt[:, r0 + dy : r0 + dy + rows, dx : dx + W],
                            start=(it == 0),
                            stop=(it == len(taps) - 1),
                            skip_group_check=True,
                        )
                    evict(pst[blk], f"ob{b}_{blk}", b, r0, rows)
    _thin_engine_sem(nc, "PE_")
    nc.finalize()
    return nc


def _thin_engine_sem(nc, sem_prefix):
    """Per-instruction semaphore increments serialize ~26ns each on the
    issuing engine. Keep only the increments that satisfy some wait
    threshold; remap wait values accordingly."""
    import concourse.mybir as mybir  # noqa: F401

    fn = nc.m.functions[0]
    insts = [i for b in fn.blocks for i in b.instructions]
    sem_ids = {}
    for i in insts:
        si = i.sync_info
        if not si:
            continue
        for u in si.on_update:
            if u.ant_name and u.ant_name.startswith(sem_prefix):
                ok = u.update_mode == "sem-inc" and u.update_value == 1
                sem_ids[u.id] = sem_ids.get(u.id, True) and ok
    for sid, ok in sem_ids.items():
        if not ok:
            continue
        thresholds = set()
        for i in insts:
            si = i.sync_info
            if not si:
                continue
            for w in si.on_wait:
                if w.id == sid:
                    if w.wait_mode != "sem-ge-imm":
                        thresholds = None
                        break
                    thresholds.add(w.wait_value)
            if thresholds is None:
                break
        if not thresholds:
            continue
        ordered = sorted(thresholds)
        rank = {k: j + 1 for j, k in enumerate(ordered)}
        count = 0
        for i in insts:
            si = i.sync_info
            if not si:
                continue
            keep = []
            for u in si.on_update:
                if u.id == sid:
                    count += 1
                    if count in thresholds:
                        keep.append(u)
                else:
                    keep.append(u)
            if len(keep) != len(si.on_update):
                si.on_update = keep
            for w in si.on_wait:
                if w.id == sid:
                    w.wait_value = rank[w.wait_value]


def _host_prep(x, routing_weight, weight, bias, mode=None):
    import ml_dtypes

    if mode in (None, "bf16", "f32r", "f32"):
        mode = MODE
    r = np.asarray(routing_weight, dtype=np.float32)
    b_mixT = np.ascontiguousarray((r @ bias).T)  # [oc, bs]
    bf = ml_dtypes.bfloat16

    if mode == "wino":
        # weight [k, oc, cin, 3, 3] -> [k, dy, dx, cin, oc]
        wt = np.ascontiguousarray(
            np.transpose(weight, (0, 3, 4, 2, 1))
        ).reshape(KEXP, -1)
        wm = (r @ wt).reshape(BS, 3, 3, CIN, OC)  # [b, dy, dx, cin, oc]
        U = np.empty((BS, 4, 3, CIN, OC), np.float32)
        U[:, 0] = wm[:, 0]
        U[:, 3] = wm[:, 2]
        U[:, 1] = 0.5 * (wm[:, 0] + wm[:, 1] + wm[:, 2])
        U[:, 2] = 0.5 * (wm[:, 0] - wm[:, 1] + wm[:, 2])
        Uf = np.ascontiguousarray(np.transpose(U, (0, 3, 1, 2, 4))).reshape(
            BS, CIN, USZ
        )
        xpad = np.zeros((BS, CIN, HP, WP), np.float32)
        xpad[:, :, 1 : H + 1, 1 : W + 1] = x
        xpad = xpad.astype(bf)
        ev = xpad[:, :, 0::2]                     # [29, WP] rows 2t
        od = xpad[:, :, 1::2]                     # [29, WP] rows 2t+1
        odf = od.astype(np.float32)
        V1 = (odf[:, :, :NT] + ev[:, :, 1 : NT + 1].astype(np.float32)).astype(bf)
        V3 = (odf[:, :, :NT] - odf[:, :, 1 : NT + 1]).astype(bf)
        evf = ev[:, :, 1 : NT + 1].astype(np.float32)
        V2 = (2.0 * evf - V1.astype(np.float32)).astype(bf)
        pk = np.zeros((BS, CIN, PKSZ_W), dtype=bf)
        pk[:, :, :USZ] = Uf
        o = USZ
        pk[:, :, o : o + 16 * WP] = V1[:, :, :16].reshape(BS, CIN, -1)
        o += 16 * WP
        pk[:, :, o : o + 29 * WP] = ev.reshape(BS, CIN, -1)
        o += 29 * WP
        pk[:, :, o : o + 16 * WP] = V3[:, :, :16].reshape(BS, CIN, -1)
        o += 16 * WP
        pk[:, :, o : o + 12 * WP] = V2[:, :, 16:].reshape(BS, CIN, -1)
        o += 12 * WP
        pk[:, :, o : o + 12 * WP] = V1[:, :, 16:].reshape(BS, CIN, -1)
        o += 12 * WP
        pk[:, :, o : o + 12 * WP] = V3[:, :, 16:].reshape(BS, CIN, -1)
    else:
        wflat = np.ascontiguousarray(
            np.transpose(weight, (0, 2, 3, 4, 1))
        ).reshape(KEXP, -1)
        w_mix = (r @ wflat).reshape(BS, CIN, WSZ)
        pk = np.zeros((BS, CIN, PKSZ_D), dtype=bf)
        pk[:, :, :WSZ] = w_mix
        xpad = pk[:, :, WSZ:].reshape(BS, CIN, HP, WP)
        xpad[:, :, 1 : H + 1, 1 : W + 1] = x

    in_maps = []
    for i in range(N_CORES):
        sl = slice(i * P, (i + 1) * P)
        in_maps.append(
            {
                "xw": np.ascontiguousarray(pk[sl]),
                "bm": np.ascontiguousarray(b_mixT[:, sl]),
            }
        )
    return in_maps


# kept for test.py compatibility (SIM path passes K.MM_DTYPE through)
MM_DTYPE = "bf16"


def _install_ntff_hook():
    """bass_utils imports antenv.axon_hooks for trace=True; the installed
    antenv lacks it. Provide it, registering the ctypes NTFF hook against
    libaxon_pjrt.so (same as trn_boot's _ntff_profile_via_ctypes)."""
    try:
        import antenv.axon_hooks  # noqa: F401

        return
    except ImportError:
        pass
    import contextlib
    import ctypes
    import sys as _sys
    import types

    hook = None
    so_path = "/opt/axon/libaxon_pjrt.so"
    if os.path.exists(so_path):
        lib = ctypes.CDLL(so_path)
        if hasattr(lib, "axon_start_nrt_profile"):
            lib.axon_start_nrt_profile.argtypes = [
                ctypes.POINTER(ctypes.c_int64),
                ctypes.c_size_t,
            ]
            lib.axon_start_nrt_profile.restype = ctypes.c_int64
            lib.axon_stop_nrt_profile.argtypes = [ctypes.c_char_p]
            lib.axon_stop_nrt_profile.restype = ctypes.c_int64

            @contextlib.contextmanager
            def _hook(output_dir, device_ids):
                import jax

                jax.devices()
                if device_ids:
                    ids = (ctypes.c_int64 * len(device_ids))(*device_ids)
                    rc = lib.axon_start_nrt_profile(ids, len(device_ids))
                else:
                    rc = lib.axon_start_nrt_profile(None, 0)
                if rc != 0:
                    raise RuntimeError(f"axon_start_nrt_profile rc={rc}")
                try:
                    yield
                finally:
                    n = lib.axon_stop_nrt_profile(str(output_dir).encode())
                    print(f"ntff profile: {n} file(s) -> {output_dir}")

            hook = _hook

    m = types.ModuleType("antenv.axon_hooks")
    m._hook = hook
    m.get_axon_ntff_profile_hook = lambda: m._hook
    m.set_axon_ntff_profile_hook = lambda h: setattr(m, "_hook", h)
    _sys.modules["antenv.axon_hooks"] = m


def _run(in_maps, **kw):
    _install_ntff_hook()
    from concourse.bass_utils import run_bass_kernel_spmd

    key = ("nc", MODE)
    if key not in _CACHE:
        _CACHE[key] = _build_nc(MODE)
    nc = _CACHE[key]
    return run_bass_kernel_spmd(nc, in_maps, list(range(N_CORES)), **kw)


def _deinterleave(o):
    # device out [P, OC, 2, NT, W] (even/odd row planes) -> [P, OC, H, W]
    o = np.asarray(o, dtype=np.float32)
    if o.ndim == 4:  # direct mode
        return o
    full = np.empty((o.shape[0], OC, H, W), np.float32)
    full[:, :, 0::2] = o[:, :, 0]
    full[:, :, 1::2] = o[:, :, 1]
    return full


def kernel(x, routing_weight, weight, bias):
    in_maps = _host_prep(x, routing_weight, weight, bias)
    res = _run(in_maps)
    return np.concatenate(
        [_deinterleave(res.results[i]["out"]) for i in range(N_CORES)],
        axis=0,
    )


# used by test.py for the profiled run
def kernel_profiled(x, routing_weight, weight, bias):
    in_maps = _host_prep(x, routing_weight, weight, bias)
    res = _run(in_maps, trace=True)
    out = np.concatenate(
        [_deinterleave(res.results[i]["out"]) for i in range(N_CORES)],
        axis=0,
    )
    return out, res


# revision 31
# speedup vs baseline: 1.0169x; 1.0169x over previous
"""CondConv2d Trainium2 kernel.

Math: per-sample conv kernel = routing-weighted sum of 8 expert kernels,
then a 3x3 (pad 1, stride 1) conv per sample, plus a routed bias.

MODE="wino": 1D Winograd F(2,3) along the height axis. Per sample the
conv is computed over 28 row-pair tiles t (output rows 2t, 2t+1):
    V0 = r[2t]-r[2t+2], V1 = r[2t+1]+r[2t+2],
    V2 = r[2t+2]-r[2t+1], V3 = r[2t+1]-r[2t+3]      (rows of padded x)
    m_i = sum_dx U[i,dx]^T V_i[.., dx:dx+56]         (PE, 12 matmuls/blk)
    y[2t]   = m0+m1+m2 + bias
    y[2t+1] = m1-m2-m3 + bias
with U0=w0, U1=(w0+w1+w2)/2, U2=(w0-w1+w2)/2, U3=w2 (host, per dx slab).
This does the 9-tap conv in 12 matmul-streams per 16 output rows instead
of 18 -> PE cycles drop 9*3136 -> 6*3136 per sample (94us -> 63us/core).
Forward transform runs on gpsimd (bf16 adds), the inverse runs on
vector (scalar_tensor_tensor with fused per-partition bias) plus one
scalar-engine PSUM->SBUF copy of m1 per block.

MODE="direct": the original 9-accumulating-matmul direct conv.

Shared structure:
  - Host computes per-sample combined (transformed) kernels, packs them
    with zero-padded x into one bf16 tensor per sample; b_mix as [oc,bs].
  - Data-parallel over batch: 8 samples per core x 8 cores.
  - PE warm-up matmuls un-throttle the HAM clock gate during head DMA.

Hardcoded shapes: x[64,128,56,56] f32, routing_weight[64,8] f32,
weight[8,128,128,3,3] f32, bias[8,128] f32 -> out[64,128,56,56] f32.
"""

import os

import numpy as np

N_CORES = 8
BS, CIN, H, W = 64, 128, 56, 56
KEXP, OC = 8, 128
P = BS // N_CORES  # samples per core
HP, WP = H + 2, W + 2
XSZ = HP * WP      # padded x free size per partition
NT = H // 2        # 28 row-pair tiles per sample
USZ = 4 * 3 * OC   # wino weight slabs (i, dx, oc)
# packed free dim: [U | ev (29 rows) | V1[0:16] | V3[0:16] | V2[16:28] | V1[16:28] | V3[16:28]]
PKSZ_W = USZ + 29 * WP + 2 * 16 * WP + 3 * 12 * WP

# direct mode sizes
WSZ = 3 * 3 * OC
PKSZ_D = WSZ + XSZ
RB = 8
NBLK = H // RB

MODE = "wino"      # "wino" or "direct"
N_WARM = 42        # warm-ups bridge until the first real matmul (~9.8us)
                   # so the HAM ramp is not reset by a PE idle gap

_CACHE = {}


def _build_nc(mode):
    if mode in ("bf16", "f32r", "f32"):  # legacy arg from test.py SIM path
        mode = MODE
    import concourse.bacc as bacc
    import concourse.mybir as mybir
    import concourse.tile as tile

    if mode == "wino":
        return _build_wino(bacc, mybir, tile)
    return _build_direct(bacc, mybir, tile)


def _build_wino(bacc, mybir, tile):
    f32 = mybir.dt.float32
    bf16 = mybir.dt.bfloat16
    Alu = mybir.AluOpType
    Act = mybir.ActivationFunctionType

    nc = bacc.Bacc()
    xw = nc.dram_tensor("xw", [P, CIN, PKSZ_W], bf16, kind="ExternalInput")
    bm = nc.dram_tensor("bm", [OC, P], f32, kind="ExternalInput")
    # even rows (y0) and odd rows (y1) in separate planes; host interleaves
    out = nc.dram_tensor("out", [P, OC, 2, NT, W], bf16, kind="ExternalOutput")

    # free-dim layout offsets (see _host_prep)
    O_EV = USZ
    O_V1A = O_EV + 29 * WP
    O_V3A = O_V1A + 16 * WP
    O_V2B = O_V3A + 16 * WP
    O_V1B = O_V2B + 12 * WP
    O_V3B = O_V1B + 12 * WP

    BLOCKS = [(0, 8), (8, 8), (16, 8), (24, 4)]

    with tile.TileContext(nc) as tc:
        with (
            tc.tile_pool(name="xp", bufs=3) as xp,
            tc.tile_pool(name="vp", bufs=3) as vp,
            tc.tile_pool(name="op", bufs=8) as op,
            tc.tile_pool(name="sp", bufs=4) as sp,
            tc.tile_pool(name="bp", bufs=1) as bp,
            tc.tile_pool(name="ps", bufs=8, space="PSUM") as psp,
        ):
            bmt = bp.tile([OC, P], f32)

            # PE warm-up (HAM un-throttle) on gpsimd-memset scratch
            scr = bp.tile([OC, 192], bf16)
            nc.gpsimd.memset(scr, 0.0)
            scrp = psp.tile([OC, 64], f32, tag="pst", name="scrp")
            for i in range(N_WARM):
                nc.tensor.matmul(
                    out=scrp[:, :],
                    lhsT=scr[:, 0:128],
                    rhs=scr[:, 128:192],
                    start=True,
                    stop=True,
                    skip_group_check=True,
                )

            xwt = [None] * P
            view = [None] * P
            vtile = [None] * P
            v2tile = [None] * P

            def dma_in(b):
                t = xp.tile([CIN, PKSZ_W], bf16, tag="xwt", name=f"xwt{b}")
                xwt[b] = t
                if b == 0:
                    # head: one sync chunk covers all of block 0's m1
                    # needs (U lhsT + V1a rhs + ev rows 0..8); V3a on the
                    # scalar ring, tail slabs on the gpsimd ring
                    c1 = O_EV + 9 * WP
                    nc.sync.dma_start(out=t[:, :c1], in_=xw[b][:, :c1])
                    nc.sync.dma_start(out=bmt, in_=bm[:, :])
                    nc.sync.dma_start(out=t[:, c1:O_V3A],
                                      in_=xw[b][:, c1:O_V3A])
                    nc.scalar.dma_start(out=t[:, O_V3A:O_V2B],
                                        in_=xw[b][:, O_V3A:O_V2B])
                    nc.gpsimd.dma_start(out=t[:, O_V2B:],
                                        in_=xw[b][:, O_V2B:])
                else:
                    nc.sync.dma_start(out=t[:, :O_V2B],
                                      in_=xw[b][:, :O_V2B])
                    nc.scalar.dma_start(out=t[:, O_V2B:],
                                        in_=xw[b][:, O_V2B:])
                view[b] = dict(
                    ut=t[:, :USZ].rearrange(
                        "p (i dx oc) -> p i dx oc", i=4, dx=3),
                    ev=t[:, O_EV:O_V1A].rearrange("p (r w) -> p r w", w=WP),
                    v1a=t[:, O_V1A:O_V3A].rearrange("p (r w) -> p r w", w=WP),
                    v3a=t[:, O_V3A:O_V2B].rearrange("p (r w) -> p r w", w=WP),
                    v2b=t[:, O_V2B:O_V1B].rearrange("p (r w) -> p r w", w=WP),
                    v1b=t[:, O_V1B:O_V3B].rearrange("p (r w) -> p r w", w=WP),
                    v3b=t[:, O_V3B:].rearrange("p (r w) -> p r w", w=WP),
                )
                vtile[b] = vp.tile([CIN, NT, WP], bf16, tag="vt",
                                   name=f"vt{b}")
                v2tile[b] = vp.tile([CIN, 16, WP], bf16, tag="v2t",
                                    name=f"v2t{b}")

            def fwd_v0(b, ts, te):
                # V0 = ev[t] - ev[t+1]  (gpsimd)
                ev = view[b]["ev"]
                nc.gpsimd.tensor_sub(
                    vtile[b][:, ts:te, :], ev[:, ts:te, :],
                    ev[:, ts + 1 : te + 1, :])

            def fwd_v2(b, ts, te):
                # V2 = 2*ev[t+1] - V1[t]  (vector; tiles 16..28 shipped)
                ev = view[b]["ev"]
                v1 = view[b]["v1a"]
                nc.vector.scalar_tensor_tensor(
                    out=v2tile[b][:, ts:te, :],
                    in0=ev[:, ts + 1 : te + 1, :], scalar=2.0,
                    in1=v1[:, ts:te, :],
                    op0=Alu.mult, op1=Alu.subtract)

            def rhs(b, i, t0, tn, dx):
                if i == 0:
                    return vtile[b][:, t0 : t0 + tn, dx : dx + W]
                if i == 2:
                    if t0 < 16:
                        return v2tile[b][:, t0 : t0 + tn, dx : dx + W]
                    return view[b]["v2b"][:, t0 - 16 : t0 - 16 + tn,
                                          dx : dx + W]
                key = ("v1a" if t0 < 16 else "v1b") if i == 1 else (
                    "v3a" if t0 < 16 else "v3b")
                o = 0 if t0 < 16 else 16
                return view[b][key][:, t0 - o : t0 - o + tn, dx : dx + W]

            def mms(b, blk, t0, tn, iorder):
                ut = view[b]["ut"]
                pst = {}
                for i in iorder:
                    ps = psp.tile([OC, tn, W], f32, tag="pst",
                                  name=f"pst{b}_{blk}_{i}")
                    pst[i] = ps
                    for dx in range(3):
                        nc.tensor.matmul(
                            out=ps[:, :, :],
                            lhsT=ut[:, i, dx, :],
                            rhs=rhs(b, i, t0, tn, dx),
                            start=(dx == 0),
                            stop=(dx == 2),
                            skip_group_check=True,
                        )
                return pst

            pending_out = [None]

            def flush_out():
                if pending_out[0] is not None:
                    d0, s0, d1, s1 = pending_out[0]
                    nc.scalar.dma_start(out=d0, in_=s0)
                    nc.sync.dma_start(out=d1, in_=s1)
                    pending_out[0] = None

            def scalar_ops(b, blk, tn, pst, tiles):
                # t1b = m1 + bias; m2f = copy(m2)   (scalar, PSUM reads)
                t1b = sp.tile([OC, tn, W], bf16, tag="t1b",
                              name=f"t1b_{b}_{blk}")
                m2f = sp.tile([OC, tn, W], bf16, tag="m2f",
                              name=f"m2f_{b}_{blk}")
                nc.scalar.activation(out=t1b, in_=pst[1][:, :, :],
                                     func=Act.Identity,
                                     bias=bmt[:, b : b + 1], scale=1.0)
                nc.scalar.activation(out=m2f, in_=pst[2][:, :, :],
                                     func=Act.Copy)
                tiles["t1b"], tiles["m2f"] = t1b, m2f

            def gpsimd_u(b, blk, tn, tiles):
                uf = sp.tile([OC, tn, W], bf16, tag="uf", name=f"u_{b}_{blk}")
                nc.gpsimd.tensor_sub(uf, tiles["t1b"], tiles["m2f"])
                tiles["uf"] = uf

            def vector_evict(b, blk, t0, tn, pst, tiles):
                s1f = sp.tile([OC, tn, W], bf16, tag="s1f",
                              name=f"s1_{b}_{blk}")
                y0t = op.tile([OC, tn, W], bf16, tag="y0t",
                              name=f"y0_{b}_{blk}")
                y1t = op.tile([OC, tn, W], bf16, tag="y1t",
                              name=f"y1_{b}_{blk}")
                nc.vector.tensor_add(s1f, tiles["t1b"], tiles["m2f"])
                nc.vector.tensor_add(y0t, pst[0][:, :, :], s1f)
                nc.vector.tensor_sub(y1t, tiles["uf"], pst[3][:, :, :])
                pending_out[0] = (out[b][:, 0, t0 : t0 + tn, :], y0t,
                                  out[b][:, 1, t0 : t0 + tn, :], y1t)

            dma_in(0)
            for b in range(P):
                if b + 1 < P:
                    dma_in(b + 1)
                blocks = BLOCKS
                nblk = len(blocks)
                if b == 0:
                    # head: the PE first streams the host-shipped V1/V3
                    # slabs of blocks 0+1 while the device computes V0/V2
                    fwd_v0(0, 0, 8)
                    fwd_v2(0, 0, 8)
                    fwd_v0(0, 8, 16)
                    fwd_v2(0, 8, 16)
                    fwd_v0(0, 16, 28)
                    psts = [dict() for _ in blocks]
                    for iorder in ((1, 3), (0, 2)):
                        for blk in (0, 1):
                            t0, tn = blocks[blk]
                            psts[blk].update(mms(0, blk, t0, tn, iorder))
                    for blk in (2, 3):
                        t0, tn = blocks[blk]
                        psts[blk] = mms(0, blk, t0, tn, (1, 2, 0, 3))
                else:
                    psts = [mms(b, blk, t0, tn, (1, 2, 0, 3))
                            for blk, (t0, tn) in enumerate(blocks)]
                tiles = [dict() for _ in range(nblk)]
                last = b == P - 1
                for blk, (t0, tn) in enumerate(blocks):
                    scalar_ops(b, blk, tn, psts[blk], tiles[blk])
                    flush_out()  # previous block's output on scalar ring
                    gpsimd_u(b, blk, tn, tiles[blk])
                    # interleave next-sample fwd into engine queues mid-way
                    if b + 1 < P:
                        if blk == 1:
                            fwd_v0(b + 1, 0, 16)
                        elif blk == 2:
                            fwd_v0(b + 1, 16, 28)
                            fwd_v2(b + 1, 0, 8)
                        elif blk == nblk - 1:
                            fwd_v2(b + 1, 8, 16)
                    vector_evict(b, blk, t0, tn, psts[blk], tiles[blk])
                    if last and blk == nblk - 1:
                        flush_out()
    _thin_engine_sem(nc, "PE_")
    nc.finalize()
    return nc


def _build_direct(bacc, mybir, tile):
    f32 = mybir.dt.float32
    bf16 = mybir.dt.bfloat16

    nc = bacc.Bacc()
    xw = nc.dram_tensor("xw", [P, CIN, PKSZ_D], bf16, kind="ExternalInput")
    bm = nc.dram_tensor("bm", [OC, P], f32, kind="ExternalInput")
    out = nc.dram_tensor("out", [P, OC, H, W], bf16, kind="ExternalOutput")

    taps = [(dy, dx) for dy in range(3) for dx in range(3)]

    with tile.TileContext(nc) as tc:
        with (
            tc.tile_pool(name="xp", bufs=4) as xp,
            tc.tile_pool(name="op", bufs=6) as op,
            tc.tile_pool(name="bp", bufs=1) as bp,
            tc.tile_pool(name="ps", bufs=8, space="PSUM") as psp,
        ):
            bmt = bp.tile([OC, P], f32)

            def evict(pst, obt_name, b, r0, rows):
                obt = op.tile([OC, rows, W], bf16, tag="obt", name=obt_name)
                nc.scalar.activation(
                    out=obt,
                    in_=pst[:, :, :],
                    func=mybir.ActivationFunctionType.Identity,
                    bias=bmt[:, b : b + 1],
                    scale=1.0,
                )
                nc.sync.dma_start(out=out[b][:, r0 : r0 + rows, :], in_=obt)

            scr = bp.tile([OC, 192], bf16)
            nc.gpsimd.memset(scr, 0.0)
            scrp = psp.tile([OC, 64], f32, tag="pst", name="scrp")
            for i in range(62):
                nc.tensor.matmul(
                    out=scrp[:, :],
                    lhsT=scr[:, 0:128],
                    rhs=scr[:, 128:192],
                    start=True,
                    stop=True,
                    skip_group_check=True,
                )

            for b in range(P):
                xwt = xp.tile([CIN, PKSZ_D], bf16)
                if b == 0:
                    c0 = WSZ + 8 * WP
                    c1 = WSZ + 10 * WP
                    c2 = WSZ + 26 * WP
                    nc.sync.dma_start(out=xwt[:, :c0], in_=xw[b][:, :c0])
                    nc.sync.dma_start(out=xwt[:, c0:c1], in_=xw[b][:, c0:c1])
                    nc.sync.dma_start(out=xwt[:, c1:c2], in_=xw[b][:, c1:c2])
                    nc.sync.dma_start(out=xwt[:, c2:], in_=xw[b][:, c2:])
                    nc.sync.dma_start(out=bmt, in_=bm[:, :])
                else:
                    nc.sync.dma_start(out=xwt, in_=xw[b])
                wt = xwt[:, :WSZ].rearrange(
                    "p (kh kw oc) -> p kh kw oc", kh=3, kw=3
                )
                xt = xwt[:, WSZ:].rearrange("p (h w) -> p h w", h=HP)
                if b == P - 1:
                    blocks = [(i * RB, RB) for i in range(NBLK - 1)]
                    blocks += [(48, 4), (52, 4)]
                else:
                    blocks = [(i * RB, RB) for i in range(NBLK)]
                pst = [
                    psp.tile([OC, rows, W], f32, tag="pst", name=f"pst{b}_{i}")
                    for i, (r0, rows) in enumerate(blocks)
                ]
                for blk, (r0, rows) in enumerate(blocks):
                    for it, (dy, dx) in enumerate(taps):
                        nc.tensor.matmul(
                            out=pst[blk][:, :, :],
                            lhsT=wt[:, dy, dx, :],
                            rhs=xt[:, r0 + dy : r0 + dy + rows, dx : dx + W],
                            start=(it == 0),
                            stop=(it == len(taps) - 1),
                            skip_group_check=True,
                        )
                    evict(pst[blk], f"ob{b}_{blk}", b, r0, rows)
    _thin_engine_sem(nc, "PE_")
    nc.finalize()
    return nc


def _thin_engine_sem(nc, sem_prefix):
    """Per-instruction semaphore increments serialize ~26ns each on the
    issuing engine. Keep only the increments that satisfy some wait
    threshold; remap wait values accordingly."""
    import concourse.mybir as mybir  # noqa: F401

    fn = nc.m.functions[0]
    insts = [i for b in fn.blocks for i in b.instructions]
    sem_ids = {}
    for i in insts:
        si = i.sync_info
        if not si:
            continue
        for u in si.on_update:
            if u.ant_name and u.ant_name.startswith(sem_prefix):
                ok = u.update_mode == "sem-inc" and u.update_value == 1
                sem_ids[u.id] = sem_ids.get(u.id, True) and ok
    for sid, ok in sem_ids.items():
        if not ok:
            continue
        thresholds = set()
        for i in insts:
            si = i.sync_info
            if not si:
                continue
            for w in si.on_wait:
                if w.id == sid:
                    if w.wait_mode != "sem-ge-imm":
                        thresholds = None
                        break
                    thresholds.add(w.wait_value)
            if thresholds is None:
                break
        if not thresholds:
            continue
        ordered = sorted(thresholds)
        rank = {k: j + 1 for j, k in enumerate(ordered)}
        count = 0
        for i in insts:
            si = i.sync_info
            if not si:
                continue
            keep = []
            for u in si.on_update:
                if u.id == sid:
                    count += 1
                    if count in thresholds:
                        keep.append(u)
                else:
                    keep.append(u)
            if len(keep) != len(si.on_update):
                si.on_update = keep
            for w in si.on_wait:
                if w.id == sid:
                    w.wait_value = rank[w.wait_value]


def _host_prep(x, routing_weight, weight, bias, mode=None):
    import ml_dtypes

    if mode in (None, "bf16", "f32r", "f32"):
        mode = MODE
    r = np.asarray(routing_weight, dtype=np.float32)
    b_mixT = np.ascontiguousarray((r @ bias).T)  # [oc, bs]
    bf = ml_dtypes.bfloat16

    if mode == "wino":
        # weight [k, oc, cin, 3, 3] -> [k, dy, dx, cin, oc]
        wt = np.ascontiguousarray(
            np.transpose(weight, (0, 3, 4, 2, 1))
        ).reshape(KEXP, -1)
        wm = (r @ wt).reshape(BS, 3, 3, CIN, OC)  # [b, dy, dx, cin, oc]
        U = np.empty((BS, 4, 3, CIN, OC), np.float32)
        U[:, 0] = wm[:, 0]
        U[:, 3] = wm[:, 2]
        U[:, 1] = 0.5 * (wm[:, 0] + wm[:, 1] + wm[:, 2])
        U[:, 2] = 0.5 * (wm[:, 0] - wm[:, 1] + wm[:, 2])
        Uf = np.ascontiguousarray(np.transpose(U, (0, 3, 1, 2, 4))).reshape(
            BS, CIN, USZ
        )
        xpad = np.zeros((BS, CIN, HP, WP), np.float32)
        xpad[:, :, 1 : H + 1, 1 : W + 1] = x
        xpad = xpad.astype(bf)
        ev = xpad[:, :, 0::2]                     # [29, WP] rows 2t
        od = xpad[:, :, 1::2]                     # [29, WP] rows 2t+1
        odf = od.astype(np.float32)
        V1 = (odf[:, :, :NT] + ev[:, :, 1 : NT + 1].astype(np.float32)).astype(bf)
        V3 = (odf[:, :, :NT] - odf[:, :, 1 : NT + 1]).astype(bf)
        evf = ev[:, :, 1 : NT + 1].astype(np.float32)
        V2 = (2.0 * evf - V1.astype(np.float32)).astype(bf)
        pk = np.zeros((BS, CIN, PKSZ_W), dtype=bf)
        pk[:, :, :USZ] = Uf
        o = USZ
        pk[:, :, o : o + 29 * WP] = ev.reshape(BS, CIN, -1)
        o += 29 * WP
        pk[:, :, o : o + 16 * WP] = V1[:, :, :16].reshape(BS, CIN, -1)
        o += 16 * WP
        pk[:, :, o : o + 16 * WP] = V3[:, :, :16].reshape(BS, CIN, -1)
        o += 16 * WP
        pk[:, :, o : o + 12 * WP] = V2[:, :, 16:].reshape(BS, CIN, -1)
        o += 12 * WP
        pk[:, :, o : o + 12 * WP] = V1[:, :, 16:].reshape(BS, CIN, -1)
        o += 12 * WP
        pk[:, :, o : o + 12 * WP] = V3[:, :, 16:].reshape(BS, CIN, -1)
    else:
        wflat = np.ascontiguousarray(
            np.transpose(weight, (0, 2, 3, 4, 1))
        ).reshape(KEXP, -1)
        w_mix = (r @ wflat).reshape(BS, CIN, WSZ)
        pk = np.zeros((BS, CIN, PKSZ_D), dtype=bf)
        pk[:, :, :WSZ] = w_mix
        xpad = pk[:, :, WSZ:].reshape(BS, CIN, HP, WP)
        xpad[:, :, 1 : H + 1, 1 : W + 1] = x

    in_maps = []
    for i in range(N_CORES):
        sl = slice(i * P, (i + 1) * P)
        in_maps.append(
            {
                "xw": np.ascontiguousarray(pk[sl]),
                "bm": np.ascontiguousarray(b_mixT[:, sl]),
            }
        )
    return in_maps


# kept for test.py compatibility (SIM path passes K.MM_DTYPE through)
MM_DTYPE = "bf16"


def _install_ntff_hook():
    """bass_utils imports antenv.axon_hooks for trace=True; the installed
    antenv lacks it. Provide it, registering the ctypes NTFF hook against
    libaxon_pjrt.so (same as trn_boot's _ntff_profile_via_ctypes)."""
    try:
        import antenv.axon_hooks  # noqa: F401

        return
    except ImportError:
        pass
    import contextlib
    import ctypes
    import sys as _sys
    import types

    hook = None
    so_path = "/opt/axon/libaxon_pjrt.so"
    if os.path.exists(so_path):
        lib = ctypes.CDLL(so_path)
        if hasattr(lib, "axon_start_nrt_profile"):
            lib.axon_start_nrt_profile.argtypes = [
                ctypes.POINTER(ctypes.c_int64),
                ctypes.c_size_t,
            ]
            lib.axon_start_nrt_profile.restype = ctypes.c_int64
            lib.axon_stop_nrt_profile.argtypes = [ctypes.c_char_p]
            lib.axon_stop_nrt_profile.restype = ctypes.c_int64

            @contextlib.contextmanager
            def _hook(output_dir, device_ids):
                import jax

                jax.devices()
                if device_ids:
                    ids = (ctypes.c_int64 * len(device_ids))(*device_ids)
                    rc = lib.axon_start_nrt_profile(ids, len(device_ids))
                else:
                    rc = lib.axon_start_nrt_profile(None, 0)
                if rc != 0:
                    raise RuntimeError(f"axon_start_nrt_profile rc={rc}")
                try:
                    yield
                finally:
                    n = lib.axon_stop_nrt_profile(str(output_dir).encode())
                    print(f"ntff profile: {n} file(s) -> {output_dir}")

            hook = _hook

    m = types.ModuleType("antenv.axon_hooks")
    m._hook = hook
    m.get_axon_ntff_profile_hook = lambda: m._hook
    m.set_axon_ntff_profile_hook = lambda h: setattr(m, "_hook", h)
    _sys.modules["antenv.axon_hooks"] = m


def _run(in_maps, **kw):
    _install_ntff_hook()
    from concourse.bass_utils import run_bass_kernel_spmd

    key = ("nc", MODE)
    if key not in _CACHE:
        _CACHE[key] = _build_nc(MODE)
    nc = _CACHE[key]
    return run_bass_kernel_spmd(nc, in_maps, list(range(N_CORES)), **kw)


def _deinterleave(o):
    # device out [P, OC, 2, NT, W] (even/odd row planes) -> [P, OC, H, W]
    o = np.asarray(o, dtype=np.float32)
    if o.ndim == 4:  # direct mode
        return o
    full = np.empty((o.shape[0], OC, H, W), np.float32)
    full[:, :, 0::2] = o[:, :, 0]
    full[:, :, 1::2] = o[:, :, 1]
    return full


def kernel(x, routing_weight, weight, bias):
    in_maps = _host_prep(x, routing_weight, weight, bias)
    res = _run(in_maps)
    return np.concatenate(
        [_deinterleave(res.results[i]["out"]) for i in range(N_CORES)],
        axis=0,
    )


# used by test.py for the profiled run
def kernel_profiled(x, routing_weight, weight, bias):
    in_maps = _host_prep(x, routing_weight, weight, bias)
    res = _run(in_maps, trace=True)
    out = np.concatenate(
        [_deinterleave(res.results[i]["out"]) for i in range(N_CORES)],
        axis=0,
    )
    return out, res


# revision 32
# speedup vs baseline: 1.0403x; 1.0230x over previous
"""CondConv2d Trainium2 kernel.

Math: per-sample conv kernel = routing-weighted sum of 8 expert kernels,
then a 3x3 (pad 1, stride 1) conv per sample, plus a routed bias.

MODE="wino": 1D Winograd F(2,3) along the height axis. Per sample the
conv is computed over 28 row-pair tiles t (output rows 2t, 2t+1):
    V0 = r[2t]-r[2t+2], V1 = r[2t+1]+r[2t+2],
    V2 = r[2t+2]-r[2t+1], V3 = r[2t+1]-r[2t+3]      (rows of padded x)
    m_i = sum_dx U[i,dx]^T V_i[.., dx:dx+56]         (PE, 12 matmuls/blk)
    y[2t]   = m0+m1+m2 + bias
    y[2t+1] = m1-m2-m3 + bias
with U0=w0, U1=(w0+w1+w2)/2, U2=(w0-w1+w2)/2, U3=w2 (host, per dx slab).
This does the 9-tap conv in 12 matmul-streams per 16 output rows instead
of 18 -> PE cycles drop 9*3136 -> 6*3136 per sample (94us -> 63us/core).
Forward transform runs on gpsimd (bf16 adds), the inverse runs on
vector (scalar_tensor_tensor with fused per-partition bias) plus one
scalar-engine PSUM->SBUF copy of m1 per block.

MODE="direct": the original 9-accumulating-matmul direct conv.

Shared structure:
  - Host computes per-sample combined (transformed) kernels, packs them
    with zero-padded x into one bf16 tensor per sample; b_mix as [oc,bs].
  - Data-parallel over batch: 8 samples per core x 8 cores.
  - PE warm-up matmuls un-throttle the HAM clock gate during head DMA.

Hardcoded shapes: x[64,128,56,56] f32, routing_weight[64,8] f32,
weight[8,128,128,3,3] f32, bias[8,128] f32 -> out[64,128,56,56] f32.
"""

import os

import numpy as np

N_CORES = 8
BS, CIN, H, W = 64, 128, 56, 56
KEXP, OC = 8, 128
P = BS // N_CORES  # samples per core
HP, WP = H + 2, W + 2
XSZ = HP * WP      # padded x free size per partition
NT = H // 2        # 28 row-pair tiles per sample
USZ = 4 * 3 * OC   # wino weight slabs (i, dx, oc)
# packed free dim: [U | ev (29 rows) | V1[0:16] | V3[0:16] | V2[16:28] | V1[16:28] | V3[16:28]]
PKSZ_W = USZ + 29 * WP + 2 * 16 * WP + 3 * 12 * WP

# direct mode sizes
WSZ = 3 * 3 * OC
PKSZ_D = WSZ + XSZ
RB = 8
NBLK = H // RB

MODE = "wino"      # "wino" or "direct"
N_WARM = 42        # warm-ups bridge until the first real matmul (~9.8us)
                   # so the HAM ramp is not reset by a PE idle gap

_CACHE = {}


def _build_nc(mode):
    if mode in ("bf16", "f32r", "f32"):  # legacy arg from test.py SIM path
        mode = MODE
    import concourse.bacc as bacc
    import concourse.mybir as mybir
    import concourse.tile as tile

    if mode == "wino":
        return _build_wino(bacc, mybir, tile)
    return _build_direct(bacc, mybir, tile)


def _build_wino(bacc, mybir, tile):
    f32 = mybir.dt.float32
    bf16 = mybir.dt.bfloat16
    Alu = mybir.AluOpType
    Act = mybir.ActivationFunctionType

    nc = bacc.Bacc()
    xw = nc.dram_tensor("xw", [P, CIN, PKSZ_W], bf16, kind="ExternalInput")
    bm = nc.dram_tensor("bm", [OC, P], f32, kind="ExternalInput")
    # even rows (y0) and odd rows (y1) in separate planes; host interleaves
    out = nc.dram_tensor("out", [P, OC, 2, NT, W], bf16, kind="ExternalOutput")

    # free-dim layout offsets (see _host_prep)
    O_EV = USZ
    O_V1A = O_EV + 29 * WP
    O_V3A = O_V1A + 16 * WP
    O_V2B = O_V3A + 16 * WP
    O_V1B = O_V2B + 12 * WP
    O_V3B = O_V1B + 12 * WP

    BLOCKS = [(0, 8), (8, 8), (16, 8), (24, 4)]

    with tile.TileContext(nc) as tc:
        with (
            tc.tile_pool(name="xp", bufs=3) as xp,
            tc.tile_pool(name="vp", bufs=3) as vp,
            tc.tile_pool(name="op", bufs=8) as op,
            tc.tile_pool(name="sp", bufs=4) as sp,
            tc.tile_pool(name="bp", bufs=1) as bp,
            tc.tile_pool(name="ps", bufs=8, space="PSUM") as psp,
        ):
            bmt = bp.tile([OC, P], f32)

            # PE warm-up (HAM un-throttle) on gpsimd-memset scratch
            scr = bp.tile([OC, 192], bf16)
            nc.gpsimd.memset(scr, 0.0)
            scrp = psp.tile([OC, 64], f32, tag="pst", name="scrp")
            for i in range(N_WARM):
                nc.tensor.matmul(
                    out=scrp[:, :],
                    lhsT=scr[:, 0:128],
                    rhs=scr[:, 128:192],
                    start=True,
                    stop=True,
                    skip_group_check=True,
                )

            xwt = [None] * P
            view = [None] * P
            vtile = [None] * P
            v2tile = [None] * P

            def dma_in(b):
                t = xp.tile([CIN, PKSZ_W], bf16, tag="xwt", name=f"xwt{b}")
                xwt[b] = t
                if b == 0:
                    # head: one sync chunk covers all of block 0's m1
                    # needs (U lhsT + V1a rhs + ev rows 0..8); V3a on the
                    # scalar ring, tail slabs on the gpsimd ring
                    c1 = O_EV + 9 * WP
                    nc.sync.dma_start(out=t[:, :c1], in_=xw[b][:, :c1])
                    nc.sync.dma_start(out=bmt, in_=bm[:, :])
                    nc.sync.dma_start(out=t[:, c1:O_V3A],
                                      in_=xw[b][:, c1:O_V3A])
                    nc.scalar.dma_start(out=t[:, O_V3A:O_V2B],
                                        in_=xw[b][:, O_V3A:O_V2B])
                    nc.gpsimd.dma_start(out=t[:, O_V2B:],
                                        in_=xw[b][:, O_V2B:])
                else:
                    nc.sync.dma_start(out=t[:, :O_V2B],
                                      in_=xw[b][:, :O_V2B])
                    nc.scalar.dma_start(out=t[:, O_V2B:],
                                        in_=xw[b][:, O_V2B:])
                view[b] = dict(
                    ut=t[:, :USZ].rearrange(
                        "p (i dx oc) -> p i dx oc", i=4, dx=3),
                    ev=t[:, O_EV:O_V1A].rearrange("p (r w) -> p r w", w=WP),
                    v1a=t[:, O_V1A:O_V3A].rearrange("p (r w) -> p r w", w=WP),
                    v3a=t[:, O_V3A:O_V2B].rearrange("p (r w) -> p r w", w=WP),
                    v2b=t[:, O_V2B:O_V1B].rearrange("p (r w) -> p r w", w=WP),
                    v1b=t[:, O_V1B:O_V3B].rearrange("p (r w) -> p r w", w=WP),
                    v3b=t[:, O_V3B:].rearrange("p (r w) -> p r w", w=WP),
                )
                vtile[b] = vp.tile([CIN, NT, WP], bf16, tag="vt",
                                   name=f"vt{b}")
                v2tile[b] = vp.tile([CIN, 16, WP], bf16, tag="v2t",
                                    name=f"v2t{b}")

            def fwd_v0(b, ts, te):
                # V0 = ev[t] - ev[t+1]  (gpsimd)
                ev = view[b]["ev"]
                nc.gpsimd.tensor_sub(
                    vtile[b][:, ts:te, :], ev[:, ts:te, :],
                    ev[:, ts + 1 : te + 1, :])

            def fwd_v2(b, ts, te):
                # V2 = 2*ev[t+1] - V1[t]  (vector; tiles 16..28 shipped)
                ev = view[b]["ev"]
                v1 = view[b]["v1a"]
                nc.vector.scalar_tensor_tensor(
                    out=v2tile[b][:, ts:te, :],
                    in0=ev[:, ts + 1 : te + 1, :], scalar=2.0,
                    in1=v1[:, ts:te, :],
                    op0=Alu.mult, op1=Alu.subtract)

            def rhs(b, i, t0, tn, dx):
                if i == 0:
                    return vtile[b][:, t0 : t0 + tn, dx : dx + W]
                if i == 2:
                    if t0 < 16:
                        return v2tile[b][:, t0 : t0 + tn, dx : dx + W]
                    return view[b]["v2b"][:, t0 - 16 : t0 - 16 + tn,
                                          dx : dx + W]
                key = ("v1a" if t0 < 16 else "v1b") if i == 1 else (
                    "v3a" if t0 < 16 else "v3b")
                o = 0 if t0 < 16 else 16
                return view[b][key][:, t0 - o : t0 - o + tn, dx : dx + W]

            def mms(b, blk, t0, tn, iorder):
                ut = view[b]["ut"]
                pst = {}
                for i in iorder:
                    ps = psp.tile([OC, tn, W], f32, tag="pst",
                                  name=f"pst{b}_{blk}_{i}")
                    pst[i] = ps
                    for dx in range(3):
                        nc.tensor.matmul(
                            out=ps[:, :, :],
                            lhsT=ut[:, i, dx, :],
                            rhs=rhs(b, i, t0, tn, dx),
                            start=(dx == 0),
                            stop=(dx == 2),
                            skip_group_check=True,
                        )
                return pst

            pending_out = [None]

            def flush_out():
                if pending_out[0] is not None:
                    d0, s0, d1, s1 = pending_out[0]
                    nc.scalar.dma_start(out=d0, in_=s0)
                    nc.sync.dma_start(out=d1, in_=s1)
                    pending_out[0] = None

            def scalar_ops(b, blk, tn, pst, tiles):
                # t1b = m1 + bias; m2f = copy(m2)   (scalar, PSUM reads)
                t1b = sp.tile([OC, tn, W], bf16, tag="t1b",
                              name=f"t1b_{b}_{blk}")
                m2f = sp.tile([OC, tn, W], bf16, tag="m2f",
                              name=f"m2f_{b}_{blk}")
                nc.scalar.activation(out=t1b, in_=pst[1][:, :, :],
                                     func=Act.Identity,
                                     bias=bmt[:, b : b + 1], scale=1.0)
                nc.scalar.activation(out=m2f, in_=pst[2][:, :, :],
                                     func=Act.Copy)
                tiles["t1b"], tiles["m2f"] = t1b, m2f

            def gpsimd_u(b, blk, tn, tiles):
                uf = sp.tile([OC, tn, W], bf16, tag="uf", name=f"u_{b}_{blk}")
                nc.gpsimd.tensor_sub(uf, tiles["t1b"], tiles["m2f"])
                tiles["uf"] = uf

            def vector_evict(b, blk, t0, tn, pst, tiles):
                s1f = sp.tile([OC, tn, W], bf16, tag="s1f",
                              name=f"s1_{b}_{blk}")
                y0t = op.tile([OC, tn, W], bf16, tag="y0t",
                              name=f"y0_{b}_{blk}")
                y1t = op.tile([OC, tn, W], bf16, tag="y1t",
                              name=f"y1_{b}_{blk}")
                nc.vector.tensor_add(s1f, tiles["t1b"], tiles["m2f"])
                nc.vector.tensor_add(y0t, pst[0][:, :, :], s1f)
                nc.vector.tensor_sub(y1t, tiles["uf"], pst[3][:, :, :])
                pending_out[0] = (out[b][:, 0, t0 : t0 + tn, :], y0t,
                                  out[b][:, 1, t0 : t0 + tn, :], y1t)

            dma_in(0)
            for b in range(P):
                if b + 1 < P:
                    dma_in(b + 1)
                blocks = BLOCKS
                nblk = len(blocks)
                if b == 0:
                    # head: the PE first streams the host-shipped V1/V3
                    # slabs of blocks 0+1 while the device computes V0/V2
                    fwd_v0(0, 0, 8)
                    fwd_v2(0, 0, 8)
                    fwd_v0(0, 8, 16)
                    fwd_v2(0, 8, 16)
                    fwd_v0(0, 16, 28)
                    psts = [dict() for _ in blocks]
                    for iorder in ((1, 3), (0, 2)):
                        for blk in (0, 1):
                            t0, tn = blocks[blk]
                            psts[blk].update(mms(0, blk, t0, tn, iorder))
                    for blk in (2, 3):
                        t0, tn = blocks[blk]
                        psts[blk] = mms(0, blk, t0, tn, (1, 2, 0, 3))
                else:
                    psts = [mms(b, blk, t0, tn, (1, 2, 0, 3))
                            for blk, (t0, tn) in enumerate(blocks)]
                tiles = [dict() for _ in range(nblk)]
                last = b == P - 1
                for blk, (t0, tn) in enumerate(blocks):
                    if last and blk == nblk - 1:
                        # short tail: no gpsimd hop; y0 is stored while
                        # y1 is still being combined
                        pst, tl = psts[blk], tiles[blk]
                        scalar_ops(b, blk, tn, pst, tl)
                        flush_out()
                        s1f = sp.tile([OC, tn, W], bf16, tag="s1f",
                                      name=f"s1_{b}_{blk}")
                        y0t = op.tile([OC, tn, W], bf16, tag="y0t",
                                      name=f"y0_{b}_{blk}")
                        y1t = op.tile([OC, tn, W], bf16, tag="y1t",
                                      name=f"y1_{b}_{blk}")
                        nc.vector.tensor_add(s1f, tl["t1b"], tl["m2f"])
                        nc.vector.tensor_add(y0t, pst[0][:, :, :], s1f)
                        nc.scalar.dma_start(
                            out=out[b][:, 0, t0 : t0 + tn, :], in_=y0t)
                        uf = sp.tile([OC, tn, W], bf16, tag="uf",
                                     name=f"u_{b}_{blk}")
                        nc.vector.tensor_sub(uf, tl["t1b"], tl["m2f"])
                        nc.vector.tensor_sub(y1t, uf, pst[3][:, :, :])
                        nc.sync.dma_start(
                            out=out[b][:, 1, t0 : t0 + tn, :], in_=y1t)
                        continue
                    scalar_ops(b, blk, tn, psts[blk], tiles[blk])
                    flush_out()  # previous block's output on scalar ring
                    gpsimd_u(b, blk, tn, tiles[blk])
                    # interleave next-sample fwd into engine queues mid-way
                    if b + 1 < P:
                        if blk == 1:
                            fwd_v0(b + 1, 0, 16)
                        elif blk == 2:
                            fwd_v0(b + 1, 16, 28)
                            fwd_v2(b + 1, 0, 8)
                        elif blk == nblk - 1:
                            fwd_v2(b + 1, 8, 16)
                    vector_evict(b, blk, t0, tn, psts[blk], tiles[blk])
    _thin_engine_sem(nc, "PE_")
    nc.finalize()
    return nc


def _build_direct(bacc, mybir, tile):
    f32 = mybir.dt.float32
    bf16 = mybir.dt.bfloat16

    nc = bacc.Bacc()
    xw = nc.dram_tensor("xw", [P, CIN, PKSZ_D], bf16, kind="ExternalInput")
    bm = nc.dram_tensor("bm", [OC, P], f32, kind="ExternalInput")
    out = nc.dram_tensor("out", [P, OC, H, W], bf16, kind="ExternalOutput")

    taps = [(dy, dx) for dy in range(3) for dx in range(3)]

    with tile.TileContext(nc) as tc:
        with (
            tc.tile_pool(name="xp", bufs=4) as xp,
            tc.tile_pool(name="op", bufs=6) as op,
            tc.tile_pool(name="bp", bufs=1) as bp,
            tc.tile_pool(name="ps", bufs=8, space="PSUM") as psp,
        ):
            bmt = bp.tile([OC, P], f32)

            def evict(pst, obt_name, b, r0, rows):
                obt = op.tile([OC, rows, W], bf16, tag="obt", name=obt_name)
                nc.scalar.activation(
                    out=obt,
                    in_=pst[:, :, :],
                    func=mybir.ActivationFunctionType.Identity,
                    bias=bmt[:, b : b + 1],
                    scale=1.0,
                )
                nc.sync.dma_start(out=out[b][:, r0 : r0 + rows, :], in_=obt)

            scr = bp.tile([OC, 192], bf16)
            nc.gpsimd.memset(scr, 0.0)
            scrp = psp.tile([OC, 64], f32, tag="pst", name="scrp")
            for i in range(62):
                nc.tensor.matmul(
                    out=scrp[:, :],
                    lhsT=scr[:, 0:128],
                    rhs=scr[:, 128:192],
                    start=True,
                    stop=True,
                    skip_group_check=True,
                )

            for b in range(P):
                xwt = xp.tile([CIN, PKSZ_D], bf16)
                if b == 0:
                    c0 = WSZ + 8 * WP
                    c1 = WSZ + 10 * WP
                    c2 = WSZ + 26 * WP
                    nc.sync.dma_start(out=xwt[:, :c0], in_=xw[b][:, :c0])
                    nc.sync.dma_start(out=xwt[:, c0:c1], in_=xw[b][:, c0:c1])
                    nc.sync.dma_start(out=xwt[:, c1:c2], in_=xw[b][:, c1:c2])
                    nc.sync.dma_start(out=xwt[:, c2:], in_=xw[b][:, c2:])
                    nc.sync.dma_start(out=bmt, in_=bm[:, :])
                else:
                    nc.sync.dma_start(out=xwt, in_=xw[b])
                wt = xwt[:, :WSZ].rearrange(
                    "p (kh kw oc) -> p kh kw oc", kh=3, kw=3
                )
                xt = xwt[:, WSZ:].rearrange("p (h w) -> p h w", h=HP)
                if b == P - 1:
                    blocks = [(i * RB, RB) for i in range(NBLK - 1)]
                    blocks += [(48, 4), (52, 4)]
                else:
                    blocks = [(i * RB, RB) for i in range(NBLK)]
                pst = [
                    psp.tile([OC, rows, W], f32, tag="pst", name=f"pst{b}_{i}")
                    for i, (r0, rows) in enumerate(blocks)
                ]
                for blk, (r0, rows) in enumerate(blocks):
                    for it, (dy, dx) in enumerate(taps):
                        nc.tensor.matmul(
                            out=pst[blk][:, :, :],
                            lhsT=wt[:, dy, dx, :],
                            rhs=xt[:, r0 + dy : r0 + dy + rows, dx : dx + W],
                            start=(it == 0),
                            stop=(it == len(taps) - 1),
                            skip_group_check=True,
                        )
                    evict(pst[blk], f"ob{b}_{blk}", b, r0, rows)
    _thin_engine_sem(nc, "PE_")
    nc.finalize()
    return nc


def _thin_engine_sem(nc, sem_prefix):
    """Per-instruction semaphore increments serialize ~26ns each on the
    issuing engine. Keep only the increments that satisfy some wait
    threshold; remap wait values accordingly."""
    import concourse.mybir as mybir  # noqa: F401

    fn = nc.m.functions[0]
    insts = [i for b in fn.blocks for i in b.instructions]
    sem_ids = {}
    for i in insts:
        si = i.sync_info
        if not si:
            continue
        for u in si.on_update:
            if u.ant_name and u.ant_name.startswith(sem_prefix):
                ok = u.update_mode == "sem-inc" and u.update_value == 1
                sem_ids[u.id] = sem_ids.get(u.id, True) and ok
    for sid, ok in sem_ids.items():
        if not ok:
            continue
        thresholds = set()
        for i in insts:
            si = i.sync_info
            if not si:
                continue
            for w in si.on_wait:
                if w.id == sid:
                    if w.wait_mode != "sem-ge-imm":
                        thresholds = None
                        break
                    thresholds.add(w.wait_value)
            if thresholds is None:
                break
        if not thresholds:
            continue
        ordered = sorted(thresholds)
        rank = {k: j + 1 for j, k in enumerate(ordered)}
        count = 0
        for i in insts:
            si = i.sync_info
            if not si:
                continue
            keep = []
            for u in si.on_update:
                if u.id == sid:
                    count += 1
                    if count in thresholds:
                        keep.append(u)
                else:
                    keep.append(u)
            if len(keep) != len(si.on_update):
                si.on_update = keep
            for w in si.on_wait:
                if w.id == sid:
                    w.wait_value = rank[w.wait_value]


def _host_prep(x, routing_weight, weight, bias, mode=None):
    import ml_dtypes

    if mode in (None, "bf16", "f32r", "f32"):
        mode = MODE
    r = np.asarray(routing_weight, dtype=np.float32)
    b_mixT = np.ascontiguousarray((r @ bias).T)  # [oc, bs]
    bf = ml_dtypes.bfloat16

    if mode == "wino":
        # weight [k, oc, cin, 3, 3] -> [k, dy, dx, cin, oc]
        wt = np.ascontiguousarray(
            np.transpose(weight, (0, 3, 4, 2, 1))
        ).reshape(KEXP, -1)
        wm = (r @ wt).reshape(BS, 3, 3, CIN, OC)  # [b, dy, dx, cin, oc]
        U = np.empty((BS, 4, 3, CIN, OC), np.float32)
        U[:, 0] = wm[:, 0]
        U[:, 3] = wm[:, 2]
        U[:, 1] = 0.5 * (wm[:, 0] + wm[:, 1] + wm[:, 2])
        U[:, 2] = 0.5 * (wm[:, 0] - wm[:, 1] + wm[:, 2])
        Uf = np.ascontiguousarray(np.transpose(U, (0, 3, 1, 2, 4))).reshape(
            BS, CIN, USZ
        )
        xpad = np.zeros((BS, CIN, HP, WP), np.float32)
        xpad[:, :, 1 : H + 1, 1 : W + 1] = x
        xpad = xpad.astype(bf)
        ev = xpad[:, :, 0::2]                     # [29, WP] rows 2t
        od = xpad[:, :, 1::2]                     # [29, WP] rows 2t+1
        odf = od.astype(np.float32)
        V1 = (odf[:, :, :NT] + ev[:, :, 1 : NT + 1].astype(np.float32)).astype(bf)
        V3 = (odf[:, :, :NT] - odf[:, :, 1 : NT + 1]).astype(bf)
        evf = ev[:, :, 1 : NT + 1].astype(np.float32)
        V2 = (2.0 * evf - V1.astype(np.float32)).astype(bf)
        pk = np.zeros((BS, CIN, PKSZ_W), dtype=bf)
        pk[:, :, :USZ] = Uf
        o = USZ
        pk[:, :, o : o + 29 * WP] = ev.reshape(BS, CIN, -1)
        o += 29 * WP
        pk[:, :, o : o + 16 * WP] = V1[:, :, :16].reshape(BS, CIN, -1)
        o += 16 * WP
        pk[:, :, o : o + 16 * WP] = V3[:, :, :16].reshape(BS, CIN, -1)
        o += 16 * WP
        pk[:, :, o : o + 12 * WP] = V2[:, :, 16:].reshape(BS, CIN, -1)
        o += 12 * WP
        pk[:, :, o : o + 12 * WP] = V1[:, :, 16:].reshape(BS, CIN, -1)
        o += 12 * WP
        pk[:, :, o : o + 12 * WP] = V3[:, :, 16:].reshape(BS, CIN, -1)
    else:
        wflat = np.ascontiguousarray(
            np.transpose(weight, (0, 2, 3, 4, 1))
        ).reshape(KEXP, -1)
        w_mix = (r @ wflat).reshape(BS, CIN, WSZ)
        pk = np.zeros((BS, CIN, PKSZ_D), dtype=bf)
        pk[:, :, :WSZ] = w_mix
        xpad = pk[:, :, WSZ:].reshape(BS, CIN, HP, WP)
        xpad[:, :, 1 : H + 1, 1 : W + 1] = x

    in_maps = []
    for i in range(N_CORES):
        sl = slice(i * P, (i + 1) * P)
        in_maps.append(
            {
                "xw": np.ascontiguousarray(pk[sl]),
                "bm": np.ascontiguousarray(b_mixT[:, sl]),
            }
        )
    return in_maps


# kept for test.py compatibility (SIM path passes K.MM_DTYPE through)
MM_DTYPE = "bf16"


def _install_ntff_hook():
    """bass_utils imports antenv.axon_hooks for trace=True; the installed
    antenv lacks it. Provide it, registering the ctypes NTFF hook against
    libaxon_pjrt.so (same as trn_boot's _ntff_profile_via_ctypes)."""
    try:
        import antenv.axon_hooks  # noqa: F401

        return
    except ImportError:
        pass
    import contextlib
    import ctypes
    import sys as _sys
    import types

    hook = None
    so_path = "/opt/axon/libaxon_pjrt.so"
    if os.path.exists(so_path):
        lib = ctypes.CDLL(so_path)
        if hasattr(lib, "axon_start_nrt_profile"):
            lib.axon_start_nrt_profile.argtypes = [
                ctypes.POINTER(ctypes.c_int64),
                ctypes.c_size_t,
            ]
            lib.axon_start_nrt_profile.restype = ctypes.c_int64
            lib.axon_stop_nrt_profile.argtypes = [ctypes.c_char_p]
            lib.axon_stop_nrt_profile.restype = ctypes.c_int64

            @contextlib.contextmanager
            def _hook(output_dir, device_ids):
                import jax

                jax.devices()
                if device_ids:
                    ids = (ctypes.c_int64 * len(device_ids))(*device_ids)
                    rc = lib.axon_start_nrt_profile(ids, len(device_ids))
                else:
                    rc = lib.axon_start_nrt_profile(None, 0)
                if rc != 0:
                    raise RuntimeError(f"axon_start_nrt_profile rc={rc}")
                try:
                    yield
                finally:
                    n = lib.axon_stop_nrt_profile(str(output_dir).encode())
                    print(f"ntff profile: {n} file(s) -> {output_dir}")

            hook = _hook

    m = types.ModuleType("antenv.axon_hooks")
    m._hook = hook
    m.get_axon_ntff_profile_hook = lambda: m._hook
    m.set_axon_ntff_profile_hook = lambda h: setattr(m, "_hook", h)
    _sys.modules["antenv.axon_hooks"] = m


def _run(in_maps, **kw):
    _install_ntff_hook()
    from concourse.bass_utils import run_bass_kernel_spmd

    key = ("nc", MODE)
    if key not in _CACHE:
        _CACHE[key] = _build_nc(MODE)
    nc = _CACHE[key]
    return run_bass_kernel_spmd(nc, in_maps, list(range(N_CORES)), **kw)


def _deinterleave(o):
    # device out [P, OC, 2, NT, W] (even/odd row planes) -> [P, OC, H, W]
    o = np.asarray(o, dtype=np.float32)
    if o.ndim == 4:  # direct mode
        return o
    full = np.empty((o.shape[0], OC, H, W), np.float32)
    full[:, :, 0::2] = o[:, :, 0]
    full[:, :, 1::2] = o[:, :, 1]
    return full


def kernel(x, routing_weight, weight, bias):
    in_maps = _host_prep(x, routing_weight, weight, bias)
    res = _run(in_maps)
    return np.concatenate(
        [_deinterleave(res.results[i]["out"]) for i in range(N_CORES)],
        axis=0,
    )


# used by test.py for the profiled run
def kernel_profiled(x, routing_weight, weight, bias):
    in_maps = _host_prep(x, routing_weight, weight, bias)
    res = _run(in_maps, trace=True)
    out = np.concatenate(
        [_deinterleave(res.results[i]["out"]) for i in range(N_CORES)],
        axis=0,
    )
    return out, res
